# revision 10
# baseline (speedup 1.0000x reference)
"""GF(2) linear block encoder c = (b @ G) mod 2 on 8 TRN2 NeuronCores.

Strategy:
  - Data-parallel: shard b rows (32768 -> 8 x 4096), replicate G.
  - Bits {0,1} are exact in fp8-e4m3 and products accumulate exactly in
    fp32 PSUM, so the GF(2) matmul is an fp8 DoubleRow matmul (K=256 per
    MM). HW floor: 216ns per 512-col DR matmul (1 col/cycle @2.4GHz),
    512 MMs/core = 110.6us of PE streaming.
  - Output is uint8 bits (ACT casts PSUM fp32 -> uint16, DVE ands with
    1 and casts to uint8), upcast to int32 on the host.
  - Head: the framework preamble ends ~6.6us; each dma_start costs
    ~0.7us of issue time, first data lands ~1us after the issue, and a
    queue moves ~0.15 GB/us with 2KB-per-partition descriptors (512B
    descriptors only reach half that, so G is repacked on the host so
    each (kp, n-half) piece is contiguous per partition). Critical
    pieces are pushed first: b chunks 0,1 on the otherwise-idle scalar
    queue, G h0 pieces kp-striped across sync/gpsimd in consumption
    order. 512-col zeroed warmup matmuls bridge ~7.2->10us and drive
    the DVFS ramp (PE runs at 1.2GHz until ~3us of sustained wide
    load); a seam warmup covers kp2/kp3 arrival jitter.
  - Tail: last two m-tiles extract per 512-col PSUM bank so the final
    chain is one quarter extract + one 64KiB DMA on emptied queues.
"""

import sys

import numpy as np

if "/opt/trn_rl_repo" not in sys.path:
    sys.path.insert(0, "/opt/trn_rl_repo")

import ml_dtypes

B_ROWS = 32768
K_MSG = 1024
N_CODE = 2048
NCORES = 8
M = B_ROWS // NCORES  # 4096 rows per core
KS = K_MSG // 128     # 8 k-subtiles of 128
KP = KS // 2          # 4 DoubleRow k-pair steps (K=256 each)
MT = M // 128         # 32 m-tiles
MC = 16               # b chunks along m (2 m-tiles each)
MCW = M // MC         # 256 rows per chunk
BG = 4                # b chunks per group tile
NBG = MC // BG        # 4 groups

F8 = ml_dtypes.float8_e4m3

_NC_CACHE = None


def _build_bass():
    import concourse.bacc as bacc
    import concourse.mybir as mybir
    from concourse import tile

    nc = bacc.Bacc("TRN2", target_bir_lowering=False, debug=False)

    # bt[p, c, s, j] = b bit for row m = c*MCW + j, k = s*128 + p
    bt = nc.dram_tensor("bt", [128, MC, KS, MCW], mybir.dt.float8e4, kind="ExternalInput")
    # g[p, kp, h, r, j] = G bit for k = (2*kp + r)*128 + p, n = h*1024 + j
    # (kp,h)-piece is contiguous per partition -> 2KB DMA descriptors
    g = nc.dram_tensor("g", [128, KP, 2, 2, 1024], mybir.dt.float8e4, kind="ExternalInput")
    c = nc.dram_tensor("c", [M, N_CODE], mybir.dt.uint8, kind="ExternalOutput")

    dr = mybir.MatmulPerfMode.DoubleRow
    NH = N_CODE // 2

    with tile.TileContext(nc) as tc:
        with (
            tc.tile_pool(name="persist", bufs=1) as persist,
            tc.tile_pool(name="psum", bufs=4, space="PSUM") as psum_pool,
            tc.tile_pool(name="mids", bufs=8) as mids,
            tc.tile_pool(name="c8s", bufs=8) as c8s,
        ):
            # g_tiles[kp][p, h, r, j]
            g_tiles = [
                persist.tile([128, 2, 2, 1024], mybir.dt.float8e4, name=f"gt{kp}", tag=f"g{kp}")
                for kp in range(KP)
            ]
            b_groups = [
                persist.tile([128, BG, KS, MCW], mybir.dt.float8e4, name=f"bg{i}", tag=f"bg{i}")
                for i in range(NBG)
            ]

            def gh(kp, h, eng):
                # one n-half of one kp pair of G (256 KiB, contiguous)
                eng.dma_start(out=g_tiles[kp][:, h], in_=g[:, kp, h])

            def bc(ch, eng):
                # one 256-row b chunk (256 KiB) feeding m-tiles 2ch, 2ch+1.
                # Int-indexed (not sliced) on both sides: the Tile subtile
                # matcher only pairs this write with bsta()'s int-indexed
                # reads when the ranks line up — a sliced write here leaves
                # every matmul with NO dependency on its b chunk (latent
                # race, inherited from the original kernel and fixed here).
                eng.dma_start(
                    out=b_groups[ch // BG][:, ch % BG],
                    in_=bt[:, ch],
                )

            # --- input pushes, consumption-ordered. Queues deliver only
            # ~110-130 KB/us each and start at different times (sync ~8.2us,
            # scalar ~9.0, gpsimd ~10.2), so the first PSUM's critical set
            # is sliced small: b chunk 0 in four 64KB s-pair pieces on
            # scalar, G h0 pieces kp0-2 on sync, kp3 on late-starting
            # gpsimd.
            for s2 in range(4):
                nc.scalar.dma_start(
                    out=b_groups[0][:, 0, 2 * s2 : 2 * s2 + 2],
                    in_=bt[:, 0, 2 * s2 : 2 * s2 + 2],
                )
            gh(0, 0, nc.sync)
            gh(1, 0, nc.sync)
            gh(3, 0, nc.gpsimd)
            bc(1, nc.scalar)
            gh(2, 0, nc.sync)
            for ch in (3, 5, 7, 9):
                bc(ch, nc.sync)
            for ch in (2, 4, 6, 8, 10):
                bc(ch, nc.gpsimd)
            gh(0, 1, nc.sync)
            gh(2, 1, nc.sync)
            gh(1, 1, nc.gpsimd)
            gh(3, 1, nc.gpsimd)
            for ch in (11, 13, 15):
                bc(ch, nc.sync)
            for ch in (12, 14):
                bc(ch, nc.gpsimd)

            # --- PE warmups: full-width 512-col matmuls on a zeroed dummy
            # tile into a dead PSUM bank. A tiny tile memsets first so the
            # earliest warmups start ~6.8us; the 512-col ones drive the
            # DVFS ramp while the first input DMAs fly.
            zw0 = persist.tile([128, 2, 128], mybir.dt.float8e4, name="zw0")
            zw = persist.tile([128, 2, 512], mybir.dt.float8e4, name="zwarm")
            nc.vector.memset(zw0, 0)
            nc.vector.memset(zw, 0)
            ps_warm = psum_pool.tile([128, NH], mybir.dt.float32, name="ps")

            def warm(cols=512):
                src = zw0 if cols <= 128 else zw
                nc.tensor.matmul(
                    ps_warm[:, 0:cols],
                    src[:, :, 0:128],
                    src[:, :, 0:cols],
                    start=True,
                    stop=True,
                    perf_mode=dr,
                )

            for _ in range(4):
                warm(64)
            for _ in range(3):
                warm(512)

            # output viewed per m-tile: m = mt*128 + p
            c_view = c.rearrange("(mt p) n -> mt p n", p=128)

            out_eng = [nc.gpsimd, nc.sync, nc.scalar]

            def bsta(mt, kp):
                mc, j = mt // 2, mt % 2
                return b_groups[mc // BG][
                    :, mc % BG, 2 * kp : 2 * kp + 2, j * 128 : (j + 1) * 128
                ]

            def grhs(kp, ph, q):
                # [128, 2, 512] moving operand: n-cols ph*1024+q*512 ..+512
                return g_tiles[kp][:, ph, :, q * 512 : (q + 1) * 512]

            def extract(mid, c8, ps, s0, s1):
                nc.scalar.activation(
                    mid[:, s0:s1], ps, mybir.ActivationFunctionType.Copy
                )
                nc.vector.tensor_scalar(
                    out=mid[:, s0:s1], in0=mid[:, s0:s1], scalar1=1,
                    scalar2=None, op0=mybir.AluOpType.bitwise_and,
                )
                nc.vector.tensor_scalar(
                    out=c8[:, s0:s1], in0=mid[:, s0:s1], scalar1=0,
                    scalar2=None, op0=mybir.AluOpType.bypass,
                )

            for ph in range(2):
                n0 = ph * NH
                for mt in range(MT):
                    quarter_mode = ph == 1 and mt >= MT - 2
                    if not quarter_mode:
                        ps = psum_pool.tile([128, NH], mybir.dt.float32, name="ps")
                        for kp in range(KP):
                            for nt in range(2):
                                nc.tensor.matmul(
                                    ps[:, nt * 512 : (nt + 1) * 512],
                                    bsta(mt, kp),
                                    grhs(kp, ph, nt),
                                    start=(kp == 0),
                                    stop=(kp == KP - 1),
                                    perf_mode=dr,
                                )
                            if ph == 0 and mt == 0 and kp == 1:
                                warm(512)  # cover kp2/kp3 arrival jitter
                        mid = mids.tile([128, NH], mybir.dt.uint16)
                        c8 = c8s.tile([128, NH], mybir.dt.uint8)
                        extract(mid, c8, ps, 0, NH)
                        # gpsimd's queue starts last and must be empty well
                        # before program end (its end-of-program DRAIN waits
                        # for the queue): keep it off the final third of the
                        # output stream.
                        if ph == 1 and mt >= 16:
                            eng = [nc.sync, nc.scalar][mt % 2]
                        else:
                            eng = out_eng[(ph * MT + mt) % 3]
                        eng.dma_start(out=c_view[mt][:, n0 : n0 + NH], in_=c8)
                    else:
                        # final two half-tiles: per-bank PSUM quarters so the
                        # tail is one 512-col extract chain + one 64 KiB DMA
                        # on queues that have gone idle
                        mid = mids.tile([128, NH], mybir.dt.uint16)
                        c8 = c8s.tile([128, NH], mybir.dt.uint8)
                        qrings = {(MT - 2, 0): nc.sync, (MT - 2, 1): nc.scalar,
                                  (MT - 1, 0): nc.sync, (MT - 1, 1): nc.scalar}
                        for nt in range(2):
                            psq = psum_pool.tile([128, 512], mybir.dt.float32, name="ps")
                            for kp in range(KP):
                                nc.tensor.matmul(
                                    psq,
                                    bsta(mt, kp),
                                    grhs(kp, ph, nt),
                                    start=(kp == 0),
                                    stop=(kp == KP - 1),
                                    perf_mode=dr,
                                )
                            s0, s1 = nt * 512, (nt + 1) * 512
                            extract(mid, c8, psq, s0, s1)
                            qrings[(mt, nt)].dma_start(
                                out=c_view[mt][:, n0 + s0 : n0 + s1],
                                in_=c8[:, s0:s1],
                            )

    nc.finalize()
    return nc


def _get_nc():
    global _NC_CACHE
    if _NC_CACHE is None:
        _NC_CACHE = _build_bass()
    return _NC_CACHE


def _pack_inputs(b, G):
    b8 = np.asarray(b).astype(np.uint8)
    G8 = np.asarray(G).astype(np.uint8)
    # g[p, kp, h, r, j]: k = (2*kp + r)*128 + p, n = h*1024 + j
    g_psn = G8.reshape(KS, 128, N_CODE).transpose(1, 0, 2)   # [p, s, n]
    g_f8 = (
        g_psn.reshape(128, KP, 2, 2, 1024)                    # [p, kp, r, h, j]
        .transpose(0, 1, 3, 2, 4)                             # [p, kp, h, r, j]
        .astype(F8, order="C")
    )
    bts = []
    for core in range(NCORES):
        sh = b8[core * M : (core + 1) * M]  # [M, K]
        # bt[p, c, s, j]: m = c*MCW + j, k = s*128 + p
        btc = sh.reshape(MC, MCW, KS, 128).transpose(3, 0, 2, 1)
        bts.append(btc.astype(F8, order="C"))
    return bts, g_f8


def kernel(b, G, trace=False, **run_kwargs):
    from concourse.bass_utils import run_bass_kernel_spmd

    nc = _get_nc()
    bts, g_f8 = _pack_inputs(b, G)
    in_maps = [{"bt": bts[i], "g": g_f8} for i in range(NCORES)]
    res = run_bass_kernel_spmd(
        nc, in_maps, core_ids=list(range(NCORES)), trace=trace, **run_kwargs
    )
    out = np.concatenate([res.results[i]["c"] for i in range(NCORES)], axis=0)
    out = out.astype(np.int32)
    if trace:
        kernel.last_results = res
    return out


kernel.last_results = None


# revision 18
# speedup vs baseline: 1.0156x; 1.0156x over previous
"""GF(2) linear block encoder c = (b @ G) mod 2 on 8 TRN2 NeuronCores.

Strategy:
  - Data-parallel: shard b rows (32768 -> 8 x 4096), replicate G.
  - Bits {0,1} are exact in fp8-e4m3 and products accumulate exactly in
    fp32 PSUM, so the GF(2) matmul is an fp8 DoubleRow matmul (K=256 per
    MM). HW floor: 216ns per 512-col DR matmul (1 col/cycle @2.4GHz),
    512 MMs/core = 110.6us of PE streaming.
  - Output is uint8 bits (ACT casts PSUM fp32 -> uint16, DVE ands with
    1 and casts to uint8), upcast to int32 on the host.
  - Head: the framework preamble ends ~6.6us; each dma_start costs
    ~0.7us of issue time, first data lands ~1us after the issue, and a
    queue moves ~0.15 GB/us with 2KB-per-partition descriptors (512B
    descriptors only reach half that, so G is repacked on the host so
    each (kp, n-half) piece is contiguous per partition). Critical
    pieces are pushed first: b chunks 0,1 on the otherwise-idle scalar
    queue, G h0 pieces kp-striped across sync/gpsimd in consumption
    order. 512-col zeroed warmup matmuls bridge ~7.2->10us and drive
    the DVFS ramp (PE runs at 1.2GHz until ~3us of sustained wide
    load); a seam warmup covers kp2/kp3 arrival jitter.
  - Tail: last two m-tiles extract per 512-col PSUM bank so the final
    chain is one quarter extract + one 64KiB DMA on emptied queues.
"""

import sys

import numpy as np

if "/opt/trn_rl_repo" not in sys.path:
    sys.path.insert(0, "/opt/trn_rl_repo")

import ml_dtypes

B_ROWS = 32768
K_MSG = 1024
N_CODE = 2048
NCORES = 8
M = B_ROWS // NCORES  # 4096 rows per core
KS = K_MSG // 128     # 8 k-subtiles of 128
KP = KS // 2          # 4 DoubleRow k-pair steps (K=256 each)
MT = M // 128         # 32 m-tiles
MC = 16               # b chunks along m (2 m-tiles each)
MCW = M // MC         # 256 rows per chunk
BG = 4                # b chunks per group tile
NBG = MC // BG        # 4 groups

F8 = ml_dtypes.float8_e4m3

_NC_CACHE = None


def _build_bass():
    import concourse.bacc as bacc
    import concourse.mybir as mybir
    from concourse import tile

    nc = bacc.Bacc("TRN2", target_bir_lowering=False, debug=False)

    # bt[p, c, s, j] = b bit for row m = c*MCW + j, k = s*128 + p
    bt = nc.dram_tensor("bt", [128, MC, KS, MCW], mybir.dt.float8e4, kind="ExternalInput")
    # g[p, kp, h, r, j] = G bit for k = (2*kp + r)*128 + p, n = h*1024 + j
    # (kp,h)-piece is contiguous per partition -> 2KB DMA descriptors
    g = nc.dram_tensor("g", [128, KP, 2, 2, 1024], mybir.dt.float8e4, kind="ExternalInput")
    c = nc.dram_tensor("c", [M, N_CODE], mybir.dt.uint8, kind="ExternalOutput")

    dr = mybir.MatmulPerfMode.DoubleRow
    NH = N_CODE // 2

    with tile.TileContext(nc) as tc:
        with (
            tc.tile_pool(name="persist", bufs=1) as persist,
            tc.tile_pool(name="psum", bufs=4, space="PSUM") as psum_pool,
            tc.tile_pool(name="mids", bufs=8) as mids,
            tc.tile_pool(name="c8s", bufs=8) as c8s,
        ):
            # g_tiles[kp][p, h, r, j]
            g_tiles = [
                persist.tile([128, 2, 2, 1024], mybir.dt.float8e4, name=f"gt{kp}", tag=f"g{kp}")
                for kp in range(KP)
            ]
            b_groups = [
                persist.tile([128, BG, KS, MCW], mybir.dt.float8e4, name=f"bg{i}", tag=f"bg{i}")
                for i in range(NBG)
            ]

            def gh(kp, h, eng):
                # one n-half of one kp pair of G (256 KiB, contiguous)
                eng.dma_start(out=g_tiles[kp][:, h], in_=g[:, kp, h])

            def bc(ch, eng):
                # one 256-row b chunk (256 KiB) feeding m-tiles 2ch, 2ch+1.
                # Int-indexed (not sliced) on both sides: the Tile subtile
                # matcher only pairs this write with bsta()'s int-indexed
                # reads when the ranks line up — a sliced write here leaves
                # every matmul with NO dependency on its b chunk (latent
                # race, inherited from the original kernel and fixed here).
                eng.dma_start(
                    out=b_groups[ch // BG][:, ch % BG],
                    in_=bt[:, ch],
                )

            # --- input pushes, consumption-ordered. Queues are descriptor-
            # rate bound (~60 x 2KB descriptors/us) and start at different
            # times (sync ~8.2us, scalar ~9.0, gpsimd ~10.2). b chunk 0 is
            # split by PARTITION (64 descriptors each, still 2KB per
            # descriptor) so it completes ~2x sooner on scalar; G h0 pieces
            # ride sync (kp0, kp1) and gpsimd (kp2, kp3) in consumption
            # order.
            nc.scalar.dma_start(out=b_groups[0][0:64, 0], in_=bt[0:64, 0])
            nc.scalar.dma_start(out=b_groups[0][64:128, 0], in_=bt[64:128, 0])
            gh(0, 0, nc.sync)
            gh(1, 0, nc.sync)
            gh(2, 0, nc.gpsimd)
            gh(3, 0, nc.gpsimd)
            bc(1, nc.scalar)
            for ch in (3, 5, 7, 9):
                bc(ch, nc.sync)
            for ch in (2, 4, 6, 8, 10):
                bc(ch, nc.gpsimd)
            gh(0, 1, nc.sync)
            gh(2, 1, nc.sync)
            gh(1, 1, nc.gpsimd)
            gh(3, 1, nc.gpsimd)
            for ch in (11, 13, 15):
                bc(ch, nc.sync)
            for ch in (12, 14):
                bc(ch, nc.gpsimd)

            # --- PE warmups: full-width 512-col matmuls on a zeroed dummy
            # tile into a dead PSUM bank. A tiny tile memsets first so the
            # earliest warmups start ~6.8us; the 512-col ones drive the
            # DVFS ramp while the first input DMAs fly.
            zw0 = persist.tile([128, 2, 128], mybir.dt.float8e4, name="zw0")
            zw = persist.tile([128, 2, 512], mybir.dt.float8e4, name="zwarm")
            nc.vector.memset(zw0, 0)
            nc.vector.memset(zw, 0)
            ps_warm = psum_pool.tile([128, NH], mybir.dt.float32, name="ps")

            def warm(cols=512):
                src = zw0 if cols <= 128 else zw
                nc.tensor.matmul(
                    ps_warm[:, 0:cols],
                    src[:, :, 0:128],
                    src[:, :, 0:cols],
                    start=True,
                    stop=True,
                    perf_mode=dr,
                )

            for _ in range(4):
                warm(64)
            for _ in range(7):
                warm(512)

            # output viewed per m-tile: m = mt*128 + p
            c_view = c.rearrange("(mt p) n -> mt p n", p=128)

            out_eng = [nc.gpsimd, nc.sync, nc.scalar]

            def bsta(mt, kp):
                mc, j = mt // 2, mt % 2
                return b_groups[mc // BG][
                    :, mc % BG, 2 * kp : 2 * kp + 2, j * 128 : (j + 1) * 128
                ]

            def grhs(kp, ph, q):
                # [128, 2, 512] moving operand: n-cols ph*1024+q*512 ..+512
                return g_tiles[kp][:, ph, :, q * 512 : (q + 1) * 512]

            def extract(mid, c8, ps, s0, s1):
                nc.scalar.activation(
                    mid[:, s0:s1], ps, mybir.ActivationFunctionType.Copy
                )
                nc.vector.tensor_scalar(
                    out=mid[:, s0:s1], in0=mid[:, s0:s1], scalar1=1,
                    scalar2=None, op0=mybir.AluOpType.bitwise_and,
                )
                nc.vector.tensor_scalar(
                    out=c8[:, s0:s1], in0=mid[:, s0:s1], scalar1=0,
                    scalar2=None, op0=mybir.AluOpType.bypass,
                )

            for ph in range(2):
                n0 = ph * NH
                for mt in range(MT):
                    quarter_mode = ph == 1 and mt >= MT - 2
                    if not quarter_mode:
                        ps = psum_pool.tile([128, NH], mybir.dt.float32, name="ps")
                        for kp in range(KP):
                            for nt in range(2):
                                nc.tensor.matmul(
                                    ps[:, nt * 512 : (nt + 1) * 512],
                                    bsta(mt, kp),
                                    grhs(kp, ph, nt),
                                    start=(kp == 0),
                                    stop=(kp == KP - 1),
                                    perf_mode=dr,
                                )
                            if ph == 0 and mt == 0 and kp < KP - 1:
                                warm(512)  # cover kp-arrival jitter
                        mid = mids.tile([128, NH], mybir.dt.uint16)
                        c8 = c8s.tile([128, NH], mybir.dt.uint8)
                        extract(mid, c8, ps, 0, NH)
                        # Late phase each queue only ships ~35-45 KB/us (HBM
                        # write contention across all 8 cores): keep the
                        # 3-way rotation to the very end so no queue builds
                        # a backlog its end-of-program DRAIN must wait out.
                        eng = out_eng[(ph * MT + mt) % 3]
                        eng.dma_start(out=c_view[mt][:, n0 : n0 + NH], in_=c8)
                    else:
                        # final two half-tiles: per-bank PSUM quarters so the
                        # tail is one 512-col extract chain + one 64 KiB DMA
                        # on queues that have gone idle
                        mid = mids.tile([128, NH], mybir.dt.uint16)
                        c8 = c8s.tile([128, NH], mybir.dt.uint8)
                        qrings = {(MT - 2, 0): nc.gpsimd, (MT - 2, 1): nc.scalar,
                                  (MT - 1, 0): nc.sync}
                        for nt in range(2):
                            psq = psum_pool.tile([128, 512], mybir.dt.float32, name="ps")
                            for kp in range(KP):
                                nc.tensor.matmul(
                                    psq,
                                    bsta(mt, kp),
                                    grhs(kp, ph, nt),
                                    start=(kp == 0),
                                    stop=(kp == KP - 1),
                                    perf_mode=dr,
                                )
                            s0, s1 = nt * 512, (nt + 1) * 512
                            extract(mid, c8, psq, s0, s1)
                            if (mt, nt) == (MT - 1, 1):
                                # very last quarter: two 32KB flights on the
                                # two emptiest queues, concurrently
                                nc.scalar.dma_start(
                                    out=c_view[mt][:, n0 + s0 : n0 + s0 + 256],
                                    in_=c8[:, s0 : s0 + 256],
                                )
                                nc.gpsimd.dma_start(
                                    out=c_view[mt][:, n0 + s0 + 256 : n0 + s1],
                                    in_=c8[:, s0 + 256 : s1],
                                )
                            else:
                                qrings[(mt, nt)].dma_start(
                                    out=c_view[mt][:, n0 + s0 : n0 + s1],
                                    in_=c8[:, s0:s1],
                                )

    nc.finalize()
    return nc


def _get_nc():
    global _NC_CACHE
    if _NC_CACHE is None:
        _NC_CACHE = _build_bass()
    return _NC_CACHE


def _pack_inputs(b, G):
    b8 = np.asarray(b).astype(np.uint8)
    G8 = np.asarray(G).astype(np.uint8)
    # g[p, kp, h, r, j]: k = (2*kp + r)*128 + p, n = h*1024 + j
    g_psn = G8.reshape(KS, 128, N_CODE).transpose(1, 0, 2)   # [p, s, n]
    g_f8 = (
        g_psn.reshape(128, KP, 2, 2, 1024)                    # [p, kp, r, h, j]
        .transpose(0, 1, 3, 2, 4)                             # [p, kp, h, r, j]
        .astype(F8, order="C")
    )
    bts = []
    for core in range(NCORES):
        sh = b8[core * M : (core + 1) * M]  # [M, K]
        # bt[p, c, s, j]: m = c*MCW + j, k = s*128 + p
        btc = sh.reshape(MC, MCW, KS, 128).transpose(3, 0, 2, 1)
        bts.append(btc.astype(F8, order="C"))
    return bts, g_f8


def kernel(b, G, trace=False, **run_kwargs):
    from concourse.bass_utils import run_bass_kernel_spmd

    nc = _get_nc()
    bts, g_f8 = _pack_inputs(b, G)
    in_maps = [{"bt": bts[i], "g": g_f8} for i in range(NCORES)]
    res = run_bass_kernel_spmd(
        nc, in_maps, core_ids=list(range(NCORES)), trace=trace, **run_kwargs
    )
    out = np.concatenate([res.results[i]["c"] for i in range(NCORES)], axis=0)
    out = out.astype(np.int32)
    if trace:
        kernel.last_results = res
    return out


kernel.last_results = None


# revision 24
# speedup vs baseline: 1.0189x; 1.0032x over previous
"""GF(2) linear block encoder c = (b @ G) mod 2 on 8 TRN2 NeuronCores.

Strategy:
  - Data-parallel: shard b rows (32768 -> 8 x 4096), replicate G.
  - Bits {0,1} are exact in fp8-e4m3 and products accumulate exactly in
    fp32 PSUM, so the GF(2) matmul is an fp8 DoubleRow matmul (K=256 per
    MM). HW floor: 216ns per 512-col DR matmul (1 col/cycle @2.4GHz),
    512 MMs/core = 110.6us of PE streaming.
  - Output is uint8 bits (ACT casts PSUM fp32 -> uint16, DVE ands with
    1 and casts to uint8), upcast to int32 on the host.
  - Head: the framework preamble ends ~6.6us; each dma_start costs
    ~0.7us of issue time, first data lands ~1us after the issue, and a
    queue moves ~0.15 GB/us with 2KB-per-partition descriptors (512B
    descriptors only reach half that, so G is repacked on the host so
    each (kp, n-half) piece is contiguous per partition). Critical
    pieces are pushed first: b chunks 0,1 on the otherwise-idle scalar
    queue, G h0 pieces kp-striped across sync/gpsimd in consumption
    order. 512-col zeroed warmup matmuls bridge ~7.2->10us and drive
    the DVFS ramp (PE runs at 1.2GHz until ~3us of sustained wide
    load); a seam warmup covers kp2/kp3 arrival jitter.
  - Tail: last two m-tiles extract per 512-col PSUM bank so the final
    chain is one quarter extract + one 64KiB DMA on emptied queues.
"""

import sys

import numpy as np

if "/opt/trn_rl_repo" not in sys.path:
    sys.path.insert(0, "/opt/trn_rl_repo")

import ml_dtypes

B_ROWS = 32768
K_MSG = 1024
N_CODE = 2048
NCORES = 8
M = B_ROWS // NCORES  # 4096 rows per core
KS = K_MSG // 128     # 8 k-subtiles of 128
KP = KS // 2          # 4 DoubleRow k-pair steps (K=256 each)
MT = M // 128         # 32 m-tiles
MC = 16               # b chunks along m (2 m-tiles each)
MCW = M // MC         # 256 rows per chunk
BG = 4                # b chunks per group tile
NBG = MC // BG        # 4 groups

F8 = ml_dtypes.float8_e4m3

_NC_CACHE = None


def _build_bass():
    import concourse.bacc as bacc
    import concourse.mybir as mybir
    from concourse import tile

    nc = bacc.Bacc("TRN2", target_bir_lowering=False, debug=False)

    # bt[p, c, s, j] = b bit for row m = c*MCW + j, k = s*128 + p
    bt = nc.dram_tensor("bt", [128, MC, KS, MCW], mybir.dt.float8e4, kind="ExternalInput")
    # g[p, kp, h, r, j] = G bit for k = (2*kp + r)*128 + p, n = h*1024 + j
    # (kp,h)-piece is contiguous per partition -> 2KB DMA descriptors
    g = nc.dram_tensor("g", [128, KP, 2, 2, 1024], mybir.dt.float8e4, kind="ExternalInput")
    c = nc.dram_tensor("c", [M, N_CODE], mybir.dt.uint8, kind="ExternalOutput")

    dr = mybir.MatmulPerfMode.DoubleRow
    NH = N_CODE // 2

    with tile.TileContext(nc) as tc:
        with (
            tc.tile_pool(name="persist", bufs=1) as persist,
            tc.tile_pool(name="psum", bufs=4, space="PSUM") as psum_pool,
            tc.tile_pool(name="mids", bufs=8) as mids,
        ):
            # g_tiles[kp][p, h, r, j]
            g_tiles = [
                persist.tile([128, 2, 2, 1024], mybir.dt.float8e4, name=f"gt{kp}", tag=f"g{kp}")
                for kp in range(KP)
            ]
            b_groups = [
                persist.tile([128, BG, KS, MCW], mybir.dt.float8e4, name=f"bg{i}", tag=f"bg{i}")
                for i in range(NBG)
            ]

            def gh(kp, h, eng):
                # one n-half of one kp pair of G (256 KiB, contiguous)
                eng.dma_start(out=g_tiles[kp][:, h], in_=g[:, kp, h])

            def bc(ch, eng):
                # one 256-row b chunk (256 KiB) feeding m-tiles 2ch, 2ch+1.
                # Int-indexed (not sliced) on both sides: the Tile subtile
                # matcher only pairs this write with bsta()'s int-indexed
                # reads when the ranks line up — a sliced write here leaves
                # every matmul with NO dependency on its b chunk (latent
                # race, inherited from the original kernel and fixed here).
                eng.dma_start(
                    out=b_groups[ch // BG][:, ch % BG],
                    in_=bt[:, ch],
                )

            # --- input pushes, consumption-ordered. Queues are descriptor-
            # rate bound (~60 x 2KB descriptors/us) and start at different
            # times (sync ~8.2us, scalar ~9.0, gpsimd ~10.2). b chunk 0 is
            # split by PARTITION (64 descriptors each, still 2KB per
            # descriptor) so it completes ~2x sooner on scalar; G h0 pieces
            # ride sync (kp0, kp1) and gpsimd (kp2, kp3) in consumption
            # order.
            nc.scalar.dma_start(out=b_groups[0][0:64, 0], in_=bt[0:64, 0])
            nc.scalar.dma_start(out=b_groups[0][64:128, 0], in_=bt[64:128, 0])
            gh(0, 0, nc.sync)
            gh(1, 0, nc.sync)
            gh(2, 0, nc.gpsimd)
            gh(3, 0, nc.gpsimd)
            bc(1, nc.scalar)
            for ch in (3, 5, 7, 9):
                bc(ch, nc.sync)
            for ch in (2, 4, 6, 8, 10):
                bc(ch, nc.gpsimd)
            gh(0, 1, nc.sync)
            gh(2, 1, nc.sync)
            gh(1, 1, nc.gpsimd)
            gh(3, 1, nc.gpsimd)
            for ch in (11, 13, 15):
                bc(ch, nc.sync)
            for ch in (12, 14):
                bc(ch, nc.gpsimd)

            # --- PE warmups: full-width 512-col matmuls on a zeroed dummy
            # tile into a dead PSUM bank. A tiny tile memsets first so the
            # earliest warmups start ~6.8us; the 512-col ones drive the
            # DVFS ramp while the first input DMAs fly.
            zw0 = persist.tile([128, 2, 128], mybir.dt.float8e4, name="zw0")
            zw = persist.tile([128, 2, 512], mybir.dt.float8e4, name="zwarm")
            nc.vector.memset(zw0, 0)
            nc.vector.memset(zw, 0)
            ps_warm = psum_pool.tile([128, NH], mybir.dt.float32, name="ps")

            def warm(cols=512):
                src = zw0 if cols <= 128 else zw
                nc.tensor.matmul(
                    ps_warm[:, 0:cols],
                    src[:, :, 0:128],
                    src[:, :, 0:cols],
                    start=True,
                    stop=True,
                    perf_mode=dr,
                )

            for _ in range(4):
                warm(64)
            for _ in range(7):
                warm(512)

            # output viewed per m-tile: m = mt*128 + p
            c_view = c.rearrange("(mt p) n -> mt p n", p=128)

            out_eng = [nc.gpsimd, nc.sync, nc.scalar]

            # full-row output staging: both n-halves of an m-tile leave as
            # ONE dma with 2KB-per-partition descriptors. Output dmas are
            # descriptor-rate bound (~55/us/queue), so 2KB descriptors
            # double the late-phase shipping rate vs per-half 1KB pieces.
            c8all = persist.tile([128, MT, N_CODE], mybir.dt.uint8, name="c8all")

            def bsta(mt, kp):
                mc, j = mt // 2, mt % 2
                return b_groups[mc // BG][
                    :, mc % BG, 2 * kp : 2 * kp + 2, j * 128 : (j + 1) * 128
                ]

            def grhs(kp, ph, q):
                # [128, 2, 512] moving operand: n-cols ph*1024+q*512 ..+512
                return g_tiles[kp][:, ph, :, q * 512 : (q + 1) * 512]

            def extract(mid, cdst, ps, m0, m1, o0, o1):
                # PSUM fp32 -> uint16 (ACT cast) -> &1 (DVE) -> uint8 row
                nc.scalar.activation(
                    mid[:, m0:m1], ps, mybir.ActivationFunctionType.Copy
                )
                nc.vector.tensor_scalar(
                    out=mid[:, m0:m1], in0=mid[:, m0:m1], scalar1=1,
                    scalar2=None, op0=mybir.AluOpType.bitwise_and,
                )
                nc.vector.tensor_scalar(
                    out=cdst[:, o0:o1], in0=mid[:, m0:m1], scalar1=0,
                    scalar2=None, op0=mybir.AluOpType.bypass,
                )

            def ship_row(mt, eng):
                # whole 2KB c row of one m-tile (256 KiB, 2KB descriptors)
                eng.dma_start(out=c_view[mt], in_=c8all[:, mt])

            def ship_row_split(mt, eng_lo, eng_hi):
                # partition-split row: two 128 KiB flights on two queues
                eng_lo.dma_start(out=c_view[mt][0:64], in_=c8all[0:64, mt])
                eng_hi.dma_start(out=c_view[mt][64:128], in_=c8all[64:128, mt])

            for ph in range(2):
                n0 = ph * NH
                for mt in range(MT):
                    quarter_mode = ph == 1 and mt >= MT - 2
                    if not quarter_mode:
                        ps = psum_pool.tile([128, NH], mybir.dt.float32, name="ps")
                        for kp in range(KP):
                            for nt in range(2):
                                nc.tensor.matmul(
                                    ps[:, nt * 512 : (nt + 1) * 512],
                                    bsta(mt, kp),
                                    grhs(kp, ph, nt),
                                    start=(kp == 0),
                                    stop=(kp == KP - 1),
                                    perf_mode=dr,
                                )
                            if ph == 0 and mt == 0 and kp < KP - 1:
                                warm(512)  # cover kp-arrival jitter
                        mid = mids.tile([128, NH], mybir.dt.uint16)
                        extract(mid, c8all[:, mt], ps, 0, NH, n0, n0 + NH)
                        if ph == 1:
                            ship_row(mt, out_eng[mt % 3])
                    else:
                        # final two m-tiles: per-bank PSUM quarters so the
                        # tail is one 512-col extract chain, and their rows
                        # leave partition-split so no single queue holds a
                        # 256 KiB piece at program end
                        mid = mids.tile([128, NH], mybir.dt.uint16)
                        for nt in range(2):
                            psq = psum_pool.tile([128, 512], mybir.dt.float32, name="ps")
                            for kp in range(KP):
                                nc.tensor.matmul(
                                    psq,
                                    bsta(mt, kp),
                                    grhs(kp, ph, nt),
                                    start=(kp == 0),
                                    stop=(kp == KP - 1),
                                    perf_mode=dr,
                                )
                            m0, m1 = nt * 512, (nt + 1) * 512
                            extract(mid, c8all[:, mt], psq, m0, m1, n0 + m0, n0 + m1)
                        if mt == MT - 2:
                            ship_row_split(mt, nc.gpsimd, nc.sync)
                        else:
                            ship_row_split(mt, nc.scalar, nc.gpsimd)

    nc.finalize()
    return nc


def _get_nc():
    global _NC_CACHE
    if _NC_CACHE is None:
        _NC_CACHE = _build_bass()
    return _NC_CACHE


def _pack_inputs(b, G):
    b8 = np.asarray(b).astype(np.uint8)
    G8 = np.asarray(G).astype(np.uint8)
    # g[p, kp, h, r, j]: k = (2*kp + r)*128 + p, n = h*1024 + j
    g_psn = G8.reshape(KS, 128, N_CODE).transpose(1, 0, 2)   # [p, s, n]
    g_f8 = (
        g_psn.reshape(128, KP, 2, 2, 1024)                    # [p, kp, r, h, j]
        .transpose(0, 1, 3, 2, 4)                             # [p, kp, h, r, j]
        .astype(F8, order="C")
    )
    bts = []
    for core in range(NCORES):
        sh = b8[core * M : (core + 1) * M]  # [M, K]
        # bt[p, c, s, j]: m = c*MCW + j, k = s*128 + p
        btc = sh.reshape(MC, MCW, KS, 128).transpose(3, 0, 2, 1)
        bts.append(btc.astype(F8, order="C"))
    return bts, g_f8


def kernel(b, G, trace=False, **run_kwargs):
    from concourse.bass_utils import run_bass_kernel_spmd

    nc = _get_nc()
    bts, g_f8 = _pack_inputs(b, G)
    in_maps = [{"bt": bts[i], "g": g_f8} for i in range(NCORES)]
    res = run_bass_kernel_spmd(
        nc, in_maps, core_ids=list(range(NCORES)), trace=trace, **run_kwargs
    )
    out = np.concatenate([res.results[i]["c"] for i in range(NCORES)], axis=0)
    out = out.astype(np.int32)
    if trace:
        kernel.last_results = res
    return out


kernel.last_results = None
